# revision 30
# baseline (speedup 1.0000x reference)
"""Differential multi-head attention on 8 Trainium2 NeuronCores.

Sharding: tensor-parallel over heads x data-parallel over batch.
Core c handles batch b = c//4 and real heads [4*(c%4), 4*(c%4)+4).
Each core computes a partial output (its 256 attention features through
the output projection); the host sums the 4 partials per batch.

Per-core dataflow (all matmuls bf16 with fp32 PSUM accumulation):
  qT/kT = W @ x.T          [feat, s] layout (feat on partitions)
  v     = x @ Wv.T         [s, feat] layout, plus a ones column per head
  ST_c  = k_c^T q_c        scores transposed: [keys, q] (keys on partitions),
                           two concurrent 32-row PE groups (comp 1/2)
  PT_c  = exp(ST_c)        on ScalarE (scores bounded ~6.5, so no
                           max-subtraction; exp never overflows)
  OT_c  = v_aug^T @ PT_c   [65, q]: rows 0-63 = P_c @ v, row 64 = rowsum r_c
                           (both components accumulate in ONE PSUM bank; the
                           bank tracker serializes writes so comp1-kt0's
                           start=True precedes everything)
  O_aug = OT^T (PE transpose, bf16) -> [q, 65]; per-q: O = O1/r1 - lam*O2/r2
  rms   = exp(-0.5*ln(ssq/64 + eps)); attn = O*rms (subln_w, 1-lam_init and
          the q scaling are folded into the weights on the host)
  out  += attnT @ Wo'      partial over this core's 256 features

The emission order software-pipelines ScalarE (exp, 270us busy) against
PE (282us busy): per 4-ktile score group, exp(g) -> fill(g+1) -> PV(g),
with per-head normalization and the per-chunk rms/output-projection
deferred 1-3 units so neither engine sees a lump of dependent work.
Modeled per-core time (TRN2 InstructionCostModel): ~340us.
"""

import math
import sys

sys.path.insert(0, "/opt/trn_rl_repo")

from contextlib import ExitStack

import ml_dtypes
import numpy as np

import concourse.bacc as bacc
import concourse.mybir as mybir
import concourse.tile as tile
from concourse.bass_utils import run_bass_kernel_spmd

# The kernel's only transcendentals are Exp and Ln; make the activation
# table-set chooser prefer the one set containing both, so a single
# ACT_TABLE_LOAD covers the whole kernel (the default order picks
# exp_and_others for Exp, forcing ~2.6us of table reloads per chunk).
_orig_get_activation_tables = bacc.get_activation_tables


def _tables_ln_exp_pinned(arch):
    # Keep dict ORDER identical (act_func_set_id is a positional index into
    # act_info.json), but remove Exp/Ln from every other set so the chooser
    # can only satisfy them from the combined set.
    t = dict(_orig_get_activation_tables(arch))
    pref = "natural_log_exp_and_others"
    if pref not in t:
        return t
    A = mybir.ActivationFunctionType
    out = {}
    for k, v in t.items():
        if k != pref:
            v = {f for f in v if f not in (A.Exp, A.Ln)}
        out[k] = v
    return out


bacc.get_activation_tables = _tables_ln_exp_pinned

F32 = mybir.dt.float32
BF16 = mybir.dt.bfloat16
ALU = mybir.AluOpType
ACT = mybir.ActivationFunctionType

E = 1024          # embed dim
S = 2048          # sequence length
B = 2             # batch
H = 16            # real heads
D = 32            # head dim (per component)
NCORES = 8
HPC = 4           # real heads per core
FPC = HPC * 2 * D  # features per core for q/k/v slices = 256
LAMBDA_INIT = 0.8 - 0.6 * math.exp(-0.3 * 12)
EPS = 1e-5

QC = 256          # query-chunk width
NQC = S // QC     # 8
NST = QC // 128   # q-subtiles per chunk
NKT = S // 128    # 16 key tiles
GROUPS = [(0, 4), (4, 8), (8, 12), (12, 16)]


def build_kernel(reps: int = 1):
    nc = bacc.Bacc("TRN2", target_bir_lowering=False, debug=False,
                   num_devices=NCORES)
    xT = nc.dram_tensor("xT", [E, S], BF16, kind="ExternalInput")
    wq = nc.dram_tensor("wq", [E, FPC], BF16, kind="ExternalInput")
    wk = nc.dram_tensor("wk", [E, FPC], BF16, kind="ExternalInput")
    wv = nc.dram_tensor("wv", [E, FPC], BF16, kind="ExternalInput")
    wo = nc.dram_tensor("wo", [FPC, E], BF16, kind="ExternalInput")
    lam = nc.dram_tensor("lam", [128, 2], F32, kind="ExternalInput")
    idf = nc.dram_tensor("idf", [128, 128], F32, kind="ExternalInput")
    idb = nc.dram_tensor("idb", [128, 128], BF16, kind="ExternalInput")
    out = nc.dram_tensor("out", [S, E], F32, kind="ExternalOutput")

    with tile.TileContext(nc) as tc, ExitStack() as ctx:
        cpool = ctx.enter_context(tc.tile_pool(name="consts", bufs=1))
        ipool = ctx.enter_context(tc.tile_pool(name="inputs", bufs=1))
        qpool = ctx.enter_context(tc.tile_pool(name="qkv", bufs=1))
        ptp = ctx.enter_context(tc.tile_pool(name="pt", bufs=2))
        wpool = ctx.enter_context(tc.tile_pool(name="work", bufs=2))
        ps_st = ctx.enter_context(tc.tile_pool(name="pst", bufs=3, space="PSUM"))
        ps_ot = ctx.enter_context(tc.tile_pool(name="pot", bufs=2, space="PSUM"))

        lamt = cpool.tile([128, 2], F32, tag="lam")
        nc.sync.dma_start(lamt[:], lam.ap())
        lam_sb = lamt[:, 0:1]
        eps_sb = lamt[:, 1:2]
        idf_sb = cpool.tile([128, 128], F32, tag="idf")
        nc.sync.dma_start(idf_sb[:], idf.ap())
        idb_sb = cpool.tile([128, 128], BF16, tag="idb")
        nc.sync.dma_start(idb_sb[:], idb.ap())

        # DMA order: per k-block, the k-projection weights then that x block,
        # so the first QKV matmuls start as soon as possible.
        wq_sb, wk_sb, wv_sb = {}, {}, {}
        x_sb = []
        for kb in range(8):
            t = ipool.tile([128, FPC], BF16, tag=f"wk{kb}", name="t")
            nc.sync.dma_start(t[:], wk.ap()[kb * 128:(kb + 1) * 128, :])
            wk_sb[kb] = t
            t = ipool.tile([128, S], BF16, tag=f"x{kb}", name="t")
            eng = (nc.sync, nc.gpsimd)[kb % 2]
            eng.dma_start(t[:], xT.ap()[kb * 128:(kb + 1) * 128, :])
            x_sb.append(t)
        for name, dram, store in (("wq", wq, wq_sb), ("wv", wv, wv_sb)):
            for kb in range(8):
                t = ipool.tile([128, FPC], BF16, tag=f"{name}{kb}", name="t")
                nc.sync.dma_start(t[:], dram.ap()[kb * 128:(kb + 1) * 128, :])
                store[kb] = t
        wo_sb = []
        for fb in range(2):
            t = ipool.tile([128, E], BF16, tag=f"wo{fb}", name="t")
            nc.sync.dma_start(t[:], wo.ap()[fb * 128:(fb + 1) * 128, :])
            wo_sb.append(t)

        for _rep in range(reps):
            # ---------------- QKV projections ----------------
            # Emission order feeds the attention pipeline ASAP: k/q block 0
            # (heads 0-1), then v (PV operand), then k/q block 1 (heads 2-3).
            qt, kt = [None, None], [None, None]
            vt = []

            def proj_qk(dname, dst_list, w_store, fb):
                t = qpool.tile([128, S], BF16, tag=f"{dname}{fb}")
                dst_list[fb] = t
                for nch in range(4):
                    ps = ps_ot.tile([128, 512], F32, tag="pot")
                    for kb in range(8):
                        nc.tensor.matmul(
                            ps[:], w_store[kb][:, fb * 128:(fb + 1) * 128],
                            x_sb[kb][:, nch * 512:(nch + 1) * 512],
                            start=(kb == 0), stop=(kb == 7))
                    nc.vector.tensor_copy(
                        t[:, nch * 512:(nch + 1) * 512], ps[:])

            def proj_v(st):
                t = qpool.tile([128, HPC * 65], BF16, tag=f"v{st}")
                vt.append(t)
                ps = ps_ot.tile([128, FPC], F32, tag="pot")
                for kb in range(8):
                    nc.tensor.matmul(
                        ps[:], x_sb[kb][:, st * 128:(st + 1) * 128],
                        wv_sb[kb][:], start=(kb == 0), stop=(kb == 7))
                tv = t.rearrange("p (h x) -> p h x", x=65)
                nc.vector.tensor_copy(
                    tv[:, :, 0:64], ps.rearrange("p (h x) -> p h x", x=64))
                nc.vector.memset(tv[:, :, 64:65], 1.0)

            proj_qk("kt", kt, wk_sb, 0)
            proj_qk("qt", qt, wq_sb, 0)
            for st in range(16):
                proj_v(st)

            # ---------------- attention ----------------
            # QC=256 so both components' PV accumulators share ONE PSUM bank
            # (ot_both). PSUM writes to one bank are serialized in emission
            # order by Tile's bank tracker, so comp1-kt0 (start=True, clears
            # the bank's has_written bits) is guaranteed first; comp2-kt0
            # writes into still-clear bits (start=False acts as overwrite).
            # Emission is software-pipelined so the scalar engine (exp, the
            # near-bottleneck) never waits: each group's exp is followed by
            # the NEXT group's score matmuls before this group's PV matmuls,
            # and normalization/output-projection are deferred until after
            # the next unit's first fill.
            qc_state = {}

            def fill_group(ctx_u, gi):
                g0, g1 = GROUPS[gi]
                stA = ps_st.tile([128, 1024], F32, tag="st")
                stB = ps_st.tile([128, 1024], F32, tag="st")
                for j in range(g1 - g0):
                    ktile = g0 + j
                    for ps_t, off in ((stA, ctx_u["off1"]), (stB, ctx_u["off2"])):
                        tp = (off, 0) if off == 96 else None
                        nc.tensor.matmul(
                            ps_t[:, j * QC:(j + 1) * QC],
                            kt[ctx_u["fb"]][off:off + 32,
                                            ktile * 128:(ktile + 1) * 128],
                            qt[ctx_u["fb"]][off:off + 32,
                                            ctx_u["qc"] * QC:(ctx_u["qc"] + 1) * QC],
                            start=True, stop=True, tile_position=tp)
                return stA, stB

            def make_normalize(ctx_u):
                ot_both = ctx_u["ot"]
                h, attn_raw, ssq = ctx_u["h"], ctx_u["araw"], ctx_u["ssq"]

                def _normalize():
                    otsb = wpool.tile([65, 2 * QC], BF16, tag="otsb")
                    nc.vector.tensor_copy(otsb[:], ot_both[:])
                    # O_aug columns strided by 68 so each PE-transpose output
                    # lands 8-byte aligned in PSUM (bf16: 136B stride).
                    oa = ps_ot.tile([128, 272], BF16, tag="pot", name="oa")
                    for c in range(2):
                        for st in range(NST):
                            nc.tensor.transpose(
                                oa[:, 68 * (2 * c + st):68 * (2 * c + st) + 65],
                                otsb[0:65, c * QC + st * 128:c * QC + (st + 1) * 128],
                                idb_sb[0:65, 0:65])
                    for st in range(NST):
                        c1o, c2o = 68 * st, 68 * (2 + st)
                        inv1 = wpool.tile([128, 1], F32, tag="inv1")
                        inv2 = wpool.tile([128, 1], F32, tag="inv2")
                        nc.vector.reciprocal(inv1[:], oa[:, c1o + 64:c1o + 65])
                        nc.vector.reciprocal(inv2[:], oa[:, c2o + 64:c2o + 65])
                        o1n = wpool.tile([128, 64], F32, tag="o1n")
                        o2n = wpool.tile([128, 64], F32, tag="o2n")
                        nc.vector.tensor_scalar_mul(
                            o1n[:], oa[:, c1o:c1o + 64], inv1[:])
                        nc.vector.tensor_scalar(
                            o2n[:], oa[:, c2o:c2o + 64],
                            inv2[:], lam_sb, op0=ALU.mult, op1=ALU.mult)
                        nc.vector.tensor_sub(
                            attn_raw[:, st, h, :], o1n[:], o2n[:])
                        sqs = wpool.tile([128, 64], F32, tag="sqs")
                        nc.vector.tensor_mul(
                            sqs[:], attn_raw[:, st, h, :],
                            attn_raw[:, st, h, :])
                        nc.vector.tensor_reduce(
                            ssq[:, st * HPC + h:st * HPC + h + 1], sqs[:],
                            axis=mybir.AxisListType.X, op=ALU.add)
                return _normalize

            def make_rms(qc, attn_raw, ssq, box):
                def _rms():
                    # rms scale = exp(-0.5 * ln(ssq/64 + eps))
                    rln = wpool.tile([128, NST * HPC], F32, tag="rln")
                    rmsi = wpool.tile([128, NST * HPC], F32, tag="rmsi")
                    nc.scalar.activation(rln[:], ssq[:], ACT.Ln,
                                         scale=1.0 / 64.0, bias=eps_sb)
                    nc.scalar.activation(rmsi[:], rln[:], ACT.Exp, scale=-0.5)
                    attn_bf = wpool.tile([128, NST, HPC, 64], BF16, tag="abf")
                    for st in range(NST):
                        for h in range(HPC):
                            nc.vector.tensor_scalar_mul(
                                attn_bf[:, st, h, :], attn_raw[:, st, h, :],
                                rmsi[:, st * HPC + h:st * HPC + h + 1])
                    box.append(attn_bf)
                return _rms

            def make_proj(qc, st, box):
                def _proj():
                    attn_bf = box[0]
                    att_flat = attn_bf.rearrange("p s h d -> p s (h d)")
                    atps = ps_ot.tile([128, 256], BF16, tag="pot")
                    nc.tensor.transpose(atps[:, 0:128],
                                        att_flat[:, st, 0:128], idb_sb[:])
                    nc.tensor.transpose(atps[:, 128:256],
                                        att_flat[:, st, 128:256], idb_sb[:])
                    at0 = wpool.tile([128, 128], BF16, tag="at0")
                    at1 = wpool.tile([128, 128], BF16, tag="at1")
                    nc.vector.tensor_copy(at0[:], atps[:, 0:128])
                    nc.vector.tensor_copy(at1[:], atps[:, 128:256])
                    row = (qc * NST + st) * 128
                    for ec in range(2):
                        ops = ps_ot.tile([128, 512], F32, tag="pot")
                        nc.tensor.matmul(
                            ops[:], at0[:],
                            wo_sb[0][:, ec * 512:(ec + 1) * 512],
                            start=True, stop=False)
                        nc.tensor.matmul(
                            ops[:], at1[:],
                            wo_sb[1][:, ec * 512:(ec + 1) * 512],
                            start=False, stop=True)
                        osb = wpool.tile([128, 512], F32, tag="osb")
                        nc.vector.tensor_copy(osb[:], ops[:])
                        nc.sync.dma_start(
                            out.ap()[row:row + 128,
                                     ec * 512:(ec + 1) * 512], osb[:])
                return _proj

            from collections import deque
            sched = deque([[] for _ in range(8)])

            def at(k, fn):
                sched[k].append(fn)

            for qc in range(NQC):
                for h in range(HPC):
                    if h == 0:
                        qc_state["araw"] = wpool.tile(
                            [128, NST, HPC, 64], F32, tag="araw", name="araw")
                        qc_state["ssq"] = wpool.tile(
                            [128, NST * HPC], F32, tag="ssq", name="ssq")
                    u = {"qc": qc, "h": h, "fb": h // 2,
                         "off1": 64 * (h % 2), "off2": 64 * (h % 2) + 32,
                         "araw": qc_state["araw"], "ssq": qc_state["ssq"]}
                    groups_st = [fill_group(u, 0)]
                    if qc == 0 and h == 1:
                        # heads 2-3 projections, overlapped with heads 0-1 exps
                        proj_qk("kt", kt, wk_sb, 1)
                        proj_qk("qt", qt, wq_sb, 1)
                    for fn in sched.popleft():
                        fn()
                    sched.append([])
                    pt1 = ptp.tile([128, NKT * QC], BF16, tag="pt1")
                    pt2 = ptp.tile([128, NKT * QC], BF16, tag="pt2")
                    u["ot"] = ps_ot.tile([65, 2 * QC], F32, tag="pot",
                                         name="ot")
                    for gi, (g0, g1) in enumerate(GROUPS):
                        w = g1 - g0
                        stA, stB = groups_st[gi]
                        nc.scalar.activation(
                            pt1[:, g0 * QC:g1 * QC], stA[:, 0:w * QC],
                            ACT.Exp)
                        nc.scalar.activation(
                            pt2[:, g0 * QC:g1 * QC], stB[:, 0:w * QC],
                            ACT.Exp)
                        if gi + 1 < len(GROUPS):
                            groups_st.append(fill_group(u, gi + 1))
                        for c, pt in ((0, pt1), (1, pt2)):
                            for j in range(g0, g1):
                                nc.tensor.matmul(
                                    u["ot"][0:65, c * QC:(c + 1) * QC],
                                    vt[j][:, h * 65:(h + 1) * 65],
                                    pt[:, j * QC:(j + 1) * QC],
                                    start=(j == 0 and c == 0),
                                    stop=(j == NKT - 1),
                                    skip_group_check=True)
                    at(0, make_normalize(u))
                    if h == HPC - 1:
                        # the rms/apply and each output-projection subtile are
                        # spread over the next units so neither the in-order
                        # scalar engine nor PE sees a lump of tail work
                        box = []
                        at(1, make_rms(qc, qc_state["araw"],
                                       qc_state["ssq"], box))
                        at(2, make_proj(qc, 0, box))
                        at(3, make_proj(qc, 1, box))
            for chunk in list(sched):
                for fn in chunk:
                    fn()
    nc.compile()
    return nc


def _prep_core_inputs(inputs, core):
    x = np.asarray(inputs["x"], np.float32)
    Wq = np.asarray(inputs["Wq"], np.float32)
    Wk = np.asarray(inputs["Wk"], np.float32)
    Wv = np.asarray(inputs["Wv"], np.float32)
    Wo = np.asarray(inputs["Wo"], np.float32)
    subln_w = np.asarray(inputs["subln_w"], np.float32)
    b, hg = core // 4, core % 4
    sl = slice(FPC * hg, FPC * (hg + 1))
    bf = ml_dtypes.bfloat16
    scaling = D ** -0.5
    lam_full = float(
        np.exp(np.sum(np.asarray(inputs["lambda_q1"], np.float64)
                      * np.asarray(inputs["lambda_k1"], np.float64)))
        - np.exp(np.sum(np.asarray(inputs["lambda_q2"], np.float64)
                        * np.asarray(inputs["lambda_k2"], np.float64)))
        + LAMBDA_INIT)
    wo_scale = (np.tile(subln_w, HPC)[:, None] * (1.0 - LAMBDA_INIT))
    return {
        "xT": np.ascontiguousarray(x[b].T).astype(bf),
        "wq": np.ascontiguousarray(Wq[sl].T * scaling).astype(bf),
        "wk": np.ascontiguousarray(Wk[sl].T).astype(bf),
        "wv": np.ascontiguousarray(Wv[sl].T).astype(bf),
        "wo": np.ascontiguousarray(Wo[:, sl].T * wo_scale).astype(bf),
        "lam": np.stack([np.full(128, lam_full, np.float32),
                         np.full(128, EPS, np.float32)], axis=1),
        "idf": np.eye(128, dtype=np.float32),
        "idb": np.eye(128, dtype=ml_dtypes.bfloat16),
    }


_CACHED = {}


def _get_kernel(reps=1):
    if reps not in _CACHED:
        _CACHED[reps] = build_kernel(reps)
    return _CACHED[reps]


def run_on_cores(inputs, reps=1):
    nc = _get_kernel(reps)
    in_maps = [_prep_core_inputs(inputs, c) for c in range(NCORES)]
    res = run_bass_kernel_spmd(nc, in_maps, core_ids=list(range(NCORES)))
    return res


def kernel(**inputs) -> np.ndarray:
    res = run_on_cores(inputs)
    out = np.zeros((B, S, E), np.float32)
    for c in range(NCORES):
        out[c // 4] += res.results[c]["out"]
    return out


# revision 32
# speedup vs baseline: 1.0225x; 1.0225x over previous
"""Differential multi-head attention on 8 Trainium2 NeuronCores.

Sharding: tensor-parallel over heads x data-parallel over batch.
Core c handles batch b = c//4 and real heads [4*(c%4), 4*(c%4)+4).
Each core computes a partial output (its 256 attention features through
the output projection); the host sums the 4 partials per batch.

Per-core dataflow (all matmuls bf16 with fp32 PSUM accumulation):
  qT/kT = W @ x.T          [feat, s] layout (feat on partitions)
  v     = x @ Wv.T         [s, feat] layout, plus a ones column per head
  ST_c  = k_c^T q_c        scores transposed: [keys, q] (keys on partitions),
                           two concurrent 32-row PE groups (comp 1/2)
  PT_c  = exp(ST_c)        on ScalarE (scores bounded ~6.5, so no
                           max-subtraction; exp never overflows)
  OT_c  = v_aug^T @ PT_c   [65, q]: rows 0-63 = P_c @ v, row 64 = rowsum r_c
                           (both components accumulate in ONE PSUM bank; the
                           bank tracker serializes writes so comp1-kt0's
                           start=True precedes everything)
  O_aug = OT^T (PE transpose, bf16) -> [q, 65]; per-q: O = O1/r1 - lam*O2/r2
  rms   = exp(-0.5*ln(ssq/64 + eps)); attn = O*rms (subln_w, 1-lam_init and
          the q scaling are folded into the weights on the host)
  out  += attnT @ Wo'      partial over this core's 256 features

The emission order software-pipelines ScalarE (exp, 270us busy) against
PE (282us busy): per 4-ktile score group, exp(g) -> fill(g+1) -> PV(g),
with per-head normalization and the per-chunk rms/output-projection
deferred 1-3 units so neither engine sees a lump of dependent work.
Modeled per-core time (TRN2 InstructionCostModel): ~340us.
"""

import math
import sys

sys.path.insert(0, "/opt/trn_rl_repo")

from contextlib import ExitStack

import ml_dtypes
import numpy as np

import concourse.bacc as bacc
import concourse.mybir as mybir
import concourse.tile as tile
from concourse.bass_utils import run_bass_kernel_spmd

# The kernel's only transcendentals are Exp and Ln; make the activation
# table-set chooser prefer the one set containing both, so a single
# ACT_TABLE_LOAD covers the whole kernel (the default order picks
# exp_and_others for Exp, forcing ~2.6us of table reloads per chunk).
_orig_get_activation_tables = bacc.get_activation_tables


def _tables_ln_exp_pinned(arch):
    # Keep dict ORDER identical (act_func_set_id is a positional index into
    # act_info.json), but remove Exp/Ln from every other set so the chooser
    # can only satisfy them from the combined set.
    t = dict(_orig_get_activation_tables(arch))
    pref = "natural_log_exp_and_others"
    if pref not in t:
        return t
    A = mybir.ActivationFunctionType
    out = {}
    for k, v in t.items():
        if k != pref:
            v = {f for f in v if f not in (A.Exp, A.Ln)}
        out[k] = v
    return out


bacc.get_activation_tables = _tables_ln_exp_pinned

F32 = mybir.dt.float32
BF16 = mybir.dt.bfloat16
ALU = mybir.AluOpType
ACT = mybir.ActivationFunctionType

E = 1024          # embed dim
S = 2048          # sequence length
B = 2             # batch
H = 16            # real heads
D = 32            # head dim (per component)
NCORES = 8
HPC = 4           # real heads per core
FPC = HPC * 2 * D  # features per core for q/k/v slices = 256
LAMBDA_INIT = 0.8 - 0.6 * math.exp(-0.3 * 12)
EPS = 1e-5

QC = 256          # query-chunk width
NQC = S // QC     # 8
NST = QC // 128   # q-subtiles per chunk
NKT = S // 128    # 16 key tiles
GROUPS = [(0, 4), (4, 8), (8, 12), (12, 16)]


def build_kernel(reps: int = 1):
    nc = bacc.Bacc("TRN2", target_bir_lowering=False, debug=False,
                   num_devices=NCORES)
    xT = nc.dram_tensor("xT", [E, S], BF16, kind="ExternalInput")
    wq = nc.dram_tensor("wq", [E, FPC], BF16, kind="ExternalInput")
    wk = nc.dram_tensor("wk", [E, FPC], BF16, kind="ExternalInput")
    wv = nc.dram_tensor("wv", [E, FPC], BF16, kind="ExternalInput")
    wo = nc.dram_tensor("wo", [FPC, E], BF16, kind="ExternalInput")
    lam = nc.dram_tensor("lam", [128, 2], F32, kind="ExternalInput")
    idf = nc.dram_tensor("idf", [128, 128], F32, kind="ExternalInput")
    idb = nc.dram_tensor("idb", [128, 128], BF16, kind="ExternalInput")
    out = nc.dram_tensor("out", [S, E], F32, kind="ExternalOutput")

    with tile.TileContext(nc) as tc, ExitStack() as ctx:
        cpool = ctx.enter_context(tc.tile_pool(name="consts", bufs=1))
        ipool = ctx.enter_context(tc.tile_pool(name="inputs", bufs=1))
        qpool = ctx.enter_context(tc.tile_pool(name="qkv", bufs=1))
        ptp = ctx.enter_context(tc.tile_pool(name="pt", bufs=2))
        wpool = ctx.enter_context(tc.tile_pool(name="work", bufs=2))
        ps_st = ctx.enter_context(tc.tile_pool(name="pst", bufs=3, space="PSUM"))
        ps_ot = ctx.enter_context(tc.tile_pool(name="pot", bufs=2, space="PSUM"))

        lamt = cpool.tile([128, 2], F32, tag="lam")
        nc.sync.dma_start(lamt[:], lam.ap())
        lam_sb = lamt[:, 0:1]
        eps_sb = lamt[:, 1:2]
        idf_sb = cpool.tile([128, 128], F32, tag="idf")
        nc.sync.dma_start(idf_sb[:], idf.ap())
        idb_sb = cpool.tile([128, 128], BF16, tag="idb")
        nc.sync.dma_start(idb_sb[:], idb.ap())

        # DMA order: per k-block, the k-projection weights then that x block,
        # so the first QKV matmuls start as soon as possible.
        wq_sb, wk_sb, wv_sb = {}, {}, {}
        x_sb = []
        for kb in range(8):
            t = ipool.tile([128, FPC], BF16, tag=f"wk{kb}", name="t")
            nc.sync.dma_start(t[:], wk.ap()[kb * 128:(kb + 1) * 128, :])
            wk_sb[kb] = t
            t = ipool.tile([128, S], BF16, tag=f"x{kb}", name="t")
            eng = (nc.sync, nc.gpsimd)[kb % 2]
            eng.dma_start(t[:], xT.ap()[kb * 128:(kb + 1) * 128, :])
            x_sb.append(t)
        for name, dram, store in (("wq", wq, wq_sb), ("wv", wv, wv_sb)):
            for kb in range(8):
                t = ipool.tile([128, FPC], BF16, tag=f"{name}{kb}", name="t")
                nc.sync.dma_start(t[:], dram.ap()[kb * 128:(kb + 1) * 128, :])
                store[kb] = t
        wo_sb = []
        for fb in range(2):
            t = ipool.tile([128, E], BF16, tag=f"wo{fb}", name="t")
            nc.sync.dma_start(t[:], wo.ap()[fb * 128:(fb + 1) * 128, :])
            wo_sb.append(t)

        for _rep in range(reps):
            # ---------------- QKV projections ----------------
            # Emission order feeds the attention pipeline ASAP: k/q block 0
            # (heads 0-1), then v (PV operand), then k/q block 1 (heads 2-3).
            qt, kt = [None, None], [None, None]
            vt = []

            def proj_qk_round(dname, dst_list, w_store, fb, nch):
                if dst_list[fb] is None:
                    dst_list[fb] = qpool.tile([128, S], BF16,
                                              tag=f"{dname}{fb}", name="t")
                t = dst_list[fb]
                ps = ps_ot.tile([128, 512], F32, tag="pot")
                for kb in range(8):
                    nc.tensor.matmul(
                        ps[:], w_store[kb][:, fb * 128:(fb + 1) * 128],
                        x_sb[kb][:, nch * 512:(nch + 1) * 512],
                        start=(kb == 0), stop=(kb == 7))
                nc.vector.tensor_copy(
                    t[:, nch * 512:(nch + 1) * 512], ps[:])

            def proj_qk(dname, dst_list, w_store, fb):
                for nch in range(4):
                    proj_qk_round(dname, dst_list, w_store, fb, nch)

            def proj_v(st):
                t = qpool.tile([128, HPC * 65], BF16, tag=f"v{st}")
                vt.append(t)
                ps = ps_ot.tile([128, FPC], F32, tag="pot")
                for kb in range(8):
                    nc.tensor.matmul(
                        ps[:], x_sb[kb][:, st * 128:(st + 1) * 128],
                        wv_sb[kb][:], start=(kb == 0), stop=(kb == 7))
                tv = t.rearrange("p (h x) -> p h x", x=65)
                nc.vector.tensor_copy(
                    tv[:, :, 0:64], ps.rearrange("p (h x) -> p h x", x=64))
                nc.vector.memset(tv[:, :, 64:65], 1.0)

            proj_qk("kt", kt, wk_sb, 0)
            proj_qk("qt", qt, wq_sb, 0)
            for st in range(16):
                proj_v(st)

            # ---------------- attention ----------------
            # QC=256 so both components' PV accumulators share ONE PSUM bank
            # (ot_both). PSUM writes to one bank are serialized in emission
            # order by Tile's bank tracker, so comp1-kt0 (start=True, clears
            # the bank's has_written bits) is guaranteed first; comp2-kt0
            # writes into still-clear bits (start=False acts as overwrite).
            # Emission is software-pipelined so the scalar engine (exp, the
            # near-bottleneck) never waits: each group's exp is followed by
            # the NEXT group's score matmuls before this group's PV matmuls,
            # and normalization/output-projection are deferred until after
            # the next unit's first fill.
            qc_state = {}

            def fill_group(ctx_u, gi):
                g0, g1 = GROUPS[gi]
                stA = ps_st.tile([128, 1024], F32, tag="st")
                stB = ps_st.tile([128, 1024], F32, tag="st")
                for j in range(g1 - g0):
                    ktile = g0 + j
                    for ps_t, off in ((stA, ctx_u["off1"]), (stB, ctx_u["off2"])):
                        tp = (off, 0) if off == 96 else None
                        nc.tensor.matmul(
                            ps_t[:, j * QC:(j + 1) * QC],
                            kt[ctx_u["fb"]][off:off + 32,
                                            ktile * 128:(ktile + 1) * 128],
                            qt[ctx_u["fb"]][off:off + 32,
                                            ctx_u["qc"] * QC:(ctx_u["qc"] + 1) * QC],
                            start=True, stop=True, tile_position=tp)
                return stA, stB

            def make_normalize(ctx_u):
                ot_both = ctx_u["ot"]
                h, attn_raw, ssq = ctx_u["h"], ctx_u["araw"], ctx_u["ssq"]

                def _normalize():
                    otsb = wpool.tile([65, 2 * QC], BF16, tag="otsb")
                    nc.vector.tensor_copy(otsb[:], ot_both[:])
                    # O_aug columns strided by 68 so each PE-transpose output
                    # lands 8-byte aligned in PSUM (bf16: 136B stride).
                    oa = ps_ot.tile([128, 272], BF16, tag="pot", name="oa")
                    for c in range(2):
                        for st in range(NST):
                            nc.tensor.transpose(
                                oa[:, 68 * (2 * c + st):68 * (2 * c + st) + 65],
                                otsb[0:65, c * QC + st * 128:c * QC + (st + 1) * 128],
                                idb_sb[0:65, 0:65])
                    for st in range(NST):
                        c1o, c2o = 68 * st, 68 * (2 + st)
                        inv1 = wpool.tile([128, 1], F32, tag="inv1")
                        inv2 = wpool.tile([128, 1], F32, tag="inv2")
                        nc.vector.reciprocal(inv1[:], oa[:, c1o + 64:c1o + 65])
                        nc.vector.reciprocal(inv2[:], oa[:, c2o + 64:c2o + 65])
                        o1n = wpool.tile([128, 64], F32, tag="o1n")
                        o2n = wpool.tile([128, 64], F32, tag="o2n")
                        nc.vector.tensor_scalar_mul(
                            o1n[:], oa[:, c1o:c1o + 64], inv1[:])
                        nc.vector.tensor_scalar(
                            o2n[:], oa[:, c2o:c2o + 64],
                            inv2[:], lam_sb, op0=ALU.mult, op1=ALU.mult)
                        nc.vector.tensor_sub(
                            attn_raw[:, st, h, :], o1n[:], o2n[:])
                        sqs = wpool.tile([128, 64], F32, tag="sqs")
                        nc.vector.tensor_mul(
                            sqs[:], attn_raw[:, st, h, :],
                            attn_raw[:, st, h, :])
                        nc.vector.tensor_reduce(
                            ssq[:, st * HPC + h:st * HPC + h + 1], sqs[:],
                            axis=mybir.AxisListType.X, op=ALU.add)
                return _normalize

            def make_rms(qc, attn_raw, ssq, box):
                def _rms():
                    # rms scale = exp(-0.5 * ln(ssq/64 + eps))
                    rln = wpool.tile([128, NST * HPC], F32, tag="rln")
                    rmsi = wpool.tile([128, NST * HPC], F32, tag="rmsi")
                    nc.scalar.activation(rln[:], ssq[:], ACT.Ln,
                                         scale=1.0 / 64.0, bias=eps_sb)
                    nc.scalar.activation(rmsi[:], rln[:], ACT.Exp, scale=-0.5)
                    attn_bf = wpool.tile([128, NST, HPC, 64], BF16, tag="abf")
                    for st in range(NST):
                        for h in range(HPC):
                            nc.vector.tensor_scalar_mul(
                                attn_bf[:, st, h, :], attn_raw[:, st, h, :],
                                rmsi[:, st * HPC + h:st * HPC + h + 1])
                    box.append(attn_bf)
                return _rms

            def make_proj(qc, st, box):
                def _proj():
                    attn_bf = box[0]
                    att_flat = attn_bf.rearrange("p s h d -> p s (h d)")
                    atps = ps_ot.tile([128, 256], BF16, tag="pot")
                    nc.tensor.transpose(atps[:, 0:128],
                                        att_flat[:, st, 0:128], idb_sb[:])
                    nc.tensor.transpose(atps[:, 128:256],
                                        att_flat[:, st, 128:256], idb_sb[:])
                    at0 = wpool.tile([128, 128], BF16, tag="at0")
                    at1 = wpool.tile([128, 128], BF16, tag="at1")
                    nc.vector.tensor_copy(at0[:], atps[:, 0:128])
                    nc.vector.tensor_copy(at1[:], atps[:, 128:256])
                    row = (qc * NST + st) * 128
                    for ec in range(2):
                        ops = ps_ot.tile([128, 512], F32, tag="pot")
                        nc.tensor.matmul(
                            ops[:], at0[:],
                            wo_sb[0][:, ec * 512:(ec + 1) * 512],
                            start=True, stop=False)
                        nc.tensor.matmul(
                            ops[:], at1[:],
                            wo_sb[1][:, ec * 512:(ec + 1) * 512],
                            start=False, stop=True)
                        osb = wpool.tile([128, 512], F32, tag="osb")
                        nc.vector.tensor_copy(osb[:], ops[:])
                        nc.sync.dma_start(
                            out.ap()[row:row + 128,
                                     ec * 512:(ec + 1) * 512], osb[:])
                return _proj

            from collections import deque
            sched = deque([[] for _ in range(10)])

            def at(k, fn):
                sched[k].append(fn)

            # Heads 0-1 over all chunks first, then heads 2-3: the heads-2/3
            # q/k projections then spread one psum-round per unit over the
            # long heads-0/1 runway (PE soaks them into its idle slack
            # instead of stalling the scalar engine in one lump).
            units = [(qc, h) for h in (0, 1) for qc in range(NQC)]
            units += [(qc, h) for qc in range(NQC) for h in (2, 3)]
            units = [units[i] for i in range(len(units))]
            fb1_rounds = (
                [("kt", kt, wk_sb, 1, nch) for nch in range(4)]
                + [("qt", qt, wq_sb, 1, nch) for nch in range(4)])
            for ui, (qc, h) in enumerate(units):
                    if qc not in qc_state:
                        qc_state[qc] = (
                            wpool.tile([128, NST, HPC, 64], F32,
                                       tag=f"araw{qc}", name="araw"),
                            wpool.tile([128, NST * HPC], F32,
                                       tag=f"ssq{qc}", name="ssq"))
                    araw_t, ssq_t = qc_state[qc]
                    u = {"qc": qc, "h": h, "fb": h // 2,
                         "off1": 64 * (h % 2), "off2": 64 * (h % 2) + 32,
                         "araw": araw_t, "ssq": ssq_t}
                    groups_st = [fill_group(u, 0)]
                    if 2 <= ui < 10 and fb1_rounds:
                        name_, dst_list, w_store, fb_, nch_ = fb1_rounds.pop(0)
                        proj_qk_round(name_, dst_list, w_store, fb_, nch_)
                    for fn in sched.popleft():
                        fn()
                    sched.append([])
                    pt1 = ptp.tile([128, NKT * QC], BF16, tag="pt1")
                    pt2 = ptp.tile([128, NKT * QC], BF16, tag="pt2")
                    u["ot"] = ps_ot.tile([65, 2 * QC], F32, tag="pot",
                                         name="ot")
                    for gi, (g0, g1) in enumerate(GROUPS):
                        w = g1 - g0
                        stA, stB = groups_st[gi]
                        nc.scalar.activation(
                            pt1[:, g0 * QC:g1 * QC], stA[:, 0:w * QC],
                            ACT.Exp)
                        nc.scalar.activation(
                            pt2[:, g0 * QC:g1 * QC], stB[:, 0:w * QC],
                            ACT.Exp)
                        if gi + 1 < len(GROUPS):
                            groups_st.append(fill_group(u, gi + 1))
                        for c, pt in ((0, pt1), (1, pt2)):
                            for j in range(g0, g1):
                                nc.tensor.matmul(
                                    u["ot"][0:65, c * QC:(c + 1) * QC],
                                    vt[j][:, h * 65:(h + 1) * 65],
                                    pt[:, j * QC:(j + 1) * QC],
                                    start=(j == 0 and c == 0),
                                    stop=(j == NKT - 1),
                                    skip_group_check=True)
                    at(0, make_normalize(u))
                    if h == HPC - 1:
                        # the rms/apply and each output-projection subtile are
                        # spread over the next units so neither the in-order
                        # scalar engine nor PE sees a lump of tail work
                        box = []
                        at(1, make_rms(qc, araw_t, ssq_t, box))
                        at(2, make_proj(qc, 0, box))
                        at(3, make_proj(qc, 1, box))
            for chunk in list(sched):
                for fn in chunk:
                    fn()
            qc_state.clear()
    nc.compile()
    return nc


def _prep_core_inputs(inputs, core):
    x = np.asarray(inputs["x"], np.float32)
    Wq = np.asarray(inputs["Wq"], np.float32)
    Wk = np.asarray(inputs["Wk"], np.float32)
    Wv = np.asarray(inputs["Wv"], np.float32)
    Wo = np.asarray(inputs["Wo"], np.float32)
    subln_w = np.asarray(inputs["subln_w"], np.float32)
    b, hg = core // 4, core % 4
    sl = slice(FPC * hg, FPC * (hg + 1))
    bf = ml_dtypes.bfloat16
    scaling = D ** -0.5
    lam_full = float(
        np.exp(np.sum(np.asarray(inputs["lambda_q1"], np.float64)
                      * np.asarray(inputs["lambda_k1"], np.float64)))
        - np.exp(np.sum(np.asarray(inputs["lambda_q2"], np.float64)
                        * np.asarray(inputs["lambda_k2"], np.float64)))
        + LAMBDA_INIT)
    wo_scale = (np.tile(subln_w, HPC)[:, None] * (1.0 - LAMBDA_INIT))
    return {
        "xT": np.ascontiguousarray(x[b].T).astype(bf),
        "wq": np.ascontiguousarray(Wq[sl].T * scaling).astype(bf),
        "wk": np.ascontiguousarray(Wk[sl].T).astype(bf),
        "wv": np.ascontiguousarray(Wv[sl].T).astype(bf),
        "wo": np.ascontiguousarray(Wo[:, sl].T * wo_scale).astype(bf),
        "lam": np.stack([np.full(128, lam_full, np.float32),
                         np.full(128, EPS, np.float32)], axis=1),
        "idf": np.eye(128, dtype=np.float32),
        "idb": np.eye(128, dtype=ml_dtypes.bfloat16),
    }


_CACHED = {}


def _get_kernel(reps=1):
    if reps not in _CACHED:
        _CACHED[reps] = build_kernel(reps)
    return _CACHED[reps]


def run_on_cores(inputs, reps=1):
    nc = _get_kernel(reps)
    in_maps = [_prep_core_inputs(inputs, c) for c in range(NCORES)]
    res = run_bass_kernel_spmd(nc, in_maps, core_ids=list(range(NCORES)))
    return res


def kernel(**inputs) -> np.ndarray:
    res = run_on_cores(inputs)
    out = np.zeros((B, S, E), np.float32)
    for c in range(NCORES):
        out[c // 4] += res.results[c]["out"]
    return out
